# revision 10
# baseline (speedup 1.0000x reference)
"""Trainium2 Bass kernel for nn_CADenseAdd (context-adaptive low-rank dense + ReLU).

Reference math (per batch row b):
    s_b   = S + context_b @ W                  # [RANK]
    out_b = relu((x_b @ U) * s_b @ V.T + bias) # [UNITS]

Sharding: data-parallel over batch B=2048 across 8 cores (256 rows/core);
U/S/V/W replicated.  All matmuls are done "transposed" so the contraction
dim always lands on SBUF partitions with zero on-device transposes:

    sT  = W_aug^T @ ctxT_aug      [RANK,  BS]  (S folded in on the host)
    xuT = U^T @ xT                [RANK,  BS]
    tT  = xuT * sT  (cast fp16)   [RANK,  BS]
    outT[um] = Vt[um] @ tT        [UNITS, BS]  (+bias, ReLU on eviction)

The host packs every operand into [128, ...] partition-major contiguous
layout; x/ctx/W/V are fp16, U is fp8-e3m4 with the scale folded into W/S
(end-to-end rel err ~1.24e-2 vs the 2e-2 gate).  PSUM accumulation is
fp32, the elementwise xu*s is fp32.  fp8 double-pumping was measured
numerically infeasible (any single e4m3 operand alone costs ~2.7e-2).

Schedule notes (v2, ~45us target; see measured-timeline derivation below):
- The measured exec window is [first useful body instruction, end of the
  LAST queue's program INCLUDING the runtime's fixed per-queue postamble
  (~7us: each queue zeroes its ~51-semaphore partition of the 256-entry
  file)].  The runtime preamble (~6.4us of barriers) is NOT counted.
- Supply is the first-half wall: ~9.7MB of inputs at the observed
  ~0.42MB/us HBM rate, with ~2.5-3.5us DMA-completion observation
  latency.  Loads stream in strict need-order: x chunks on the SP ring
  and U chunks on the ACT ring (parallel descgen), then ctx+W, x tail,
  V.  mm1 (ctx@W) runs LATE - after a 24-kn mm2 head - so its operands
  arrive with margin (the old early-mm1 schedule stalled the PE 1.35us).
- Garbage warmup matmuls from the first PE slot release the HAM
  clock-gate (1.2 -> 2.4 GHz after ~3.9us of dense PE activity), timed
  to end right as the first x/U chunks' completions are observed.
- Bass's init-time const memsets + all-engine barrier are suppressed
  (the ACT ReLU bias reads an explicitly zeroed tile instead), starting
  the body ~1.2us earlier.
- Teardown is minimal: a chain of single-wait SP nops observes every
  proc's final tick (so the SP queue - whose semaphore partition holds
  ALL of this program's semaphores, re-homed to [207,256) - ends last),
  then nothing: no barrier, no in-program semaphore clears.  The
  runtime's own postamble DRAINs each queue's DMA rings and zeroes every
  semaphore, and the early-finishing queues' postambles overlap the PE
  stream instead of serializing after it.
"""

import re

import ml_dtypes
import numpy as np

import bass_rust
import concourse.bass as bass
import concourse.tile as tile
from concourse import mybir
from concourse.bass_utils import run_bass_kernel_spmd
from concourse.vector_clock import ScopedClock


def _minimal_drain_and_barrier(self, tick_clock, wait_clock):
    """Replacement for TileContext._drain_and_barrier.

    Emit one single-wait SP nop per active proc (this walrus build cannot
    encode >1 sync wait per instruction), so the SP queue observes every
    proc's final tick - including the output stores' DMA-lane semaphores -
    before its program ends.  Everything else (cross-engine barrier,
    in-program semaphore clears, DMA ring resets) is omitted: the runtime's
    per-queue postamble DRAINs the rings and zeroes the entire semaphore
    file anyway, and omitting the barrier lets the other queues' fixed
    ~7us postambles run concurrently with the tail of the compute stream.
    All program semaphores live in the SP queue's clear-partition
    [207, 256), so no other queue's postamble can zero a live semaphore.
    """
    ticks = [int(x) for x in re.findall(r"\d+", repr(tick_clock.global_clock))]
    for proc, tick in enumerate(ticks):
        if tick > 0:
            nop_inst = self.nc.sync.nop(nofuse=True)
            sub = bass_rust.VectorClock()
            sub.require_at_least(proc, tick)
            wait_clock.add_sem_waits(nop_inst.ins, ScopedClock({None: sub}))
    # Python-side bookkeeping of clear_and_free_semaphores, minus the
    # gpsimd dma_reset/sem_clear instructions.
    popped = self.nc._tile_sem_poison_stack.pop()
    assert popped is self._sem_poison
    sems = list(self.sems.allocated().values())
    nums = [s.num if hasattr(s, "num") else int(s) for s in sems]
    self.nc._state.prepend_free_semaphores(nums)
    for poison in self.nc._tile_sem_poison_stack:
        poison.update(nums)


tile.TileContext._drain_and_barrier = _minimal_drain_and_barrier

# Problem shape (hardcoded per contract)
M = 8  # cores
B, N, C = 2048, 4096, 1024
UNITS, RANK = 4096, 512
BS = B // M  # 256 rows per core
P = 128
KN = N // P      # 32 contraction tiles for x @ U
KC = C // P      # 8 contraction tiles for ctx @ W
RM = RANK // P   # 4 tiles of RANK
UM = UNITS // P  # 32 tiles of UNITS

F16 = mybir.dt.float16
F32 = mybir.dt.float32
F8 = mybir.dt.float8e3  # e3m4: 4 mantissa bits

# U ships as fp8-e3m4 scaled by USCALE (entries ~N(0,1/64) land in e3m4's
# normal range); the inverse scale is folded into W and S on the host, so
# tT = (64*xu) * (s/64) is exact.  Halves U's DMA bytes (4MB -> 2MB per
# core) on the supply-bound pre-mm2 stream; end-to-end metric ~1.24e-2
# (deterministic for the harness seed) vs the 2e-2 gate.
USCALE = 64.0

# Warmups bridge [body start ~6.4us, first x/U chunk observed ~10.3-11.4us]
# at the cold 1.2GHz rate (~211ns each) and release the HAM clock-gate.
# Undershooting leaves a PE idle gap that PAUSES the HAM ramp accumulation
# (measured: 20 warmups -> full clock at 12.4us vs 24 -> 11.5us).
N_WARM_MM = 21

# mm2 head length (kn tiles) before mm1 slots in: ctx+W ride the supply
# stream right behind the head's x/U chunks (0.125*18+1.69 MB at the
# ~0.42MB/us pipe from ~8.3us + ~1.3us completion observation = ~19us),
# so mm1 starts exactly as the head's PE work drains and fills the
# supply stall before the x/U tail chunk is observed.
MM2_HEAD = 18


def _make_bass() -> bass.Bass:
    """Bass with the init-time const-tile memsets and all-engine barrier
    suppressed (nothing in this program reads the const tiles - the ACT
    ReLU bias uses an explicit zeroed tile), and the semaphore free-pool
    re-homed into [207, 256): the partition of the semaphore file that the
    runtime postamble of the SP queue (the last queue to finish) clears.
    """
    orig_memset = bass.BassEitherVectorEngine.memset
    orig_aeb = bass.Bass.all_engine_barrier

    def _no_memset(self, ap, constant):
        return None

    def _no_aeb(self, sem_only=False):
        return None

    bass.BassEitherVectorEngine.memset = _no_memset
    bass.Bass.all_engine_barrier = _no_aeb
    try:
        nc = bass.Bass(
            "TRN2",
            debug=False,
            enable_asserts=False,
            enable_partition_id=False,
            dynamic_dma_scratch_size=4096,
        )
    finally:
        bass.BassEitherVectorEngine.memset = orig_memset
        bass.Bass.all_engine_barrier = orig_aeb
    nc._state.reset_free_semaphores(list(range(207, 256)))
    return nc


def build_program(zero_bias: bool = True, sim_memset: bool = False) -> bass.Bass:
    """Build the per-core SPMD program.

    Wait-encoding constraint: this walrus build cannot encode >1 sem-wait
    on DVE/ACT tensor instructions, while matmuls can encode 2.  Every
    DVE/ACT instruction keeps <=1 wait: each engine "pre-touches" its
    DMA-sourced operands once (so later instructions only wait on PE),
    PSUM banks are never shared across phases, and output staging tiles
    are never reused.
    """
    nc = _make_bass()

    # S is folded into mm1 on the host: ctxT/W carry an extra contraction
    # tile (ones-row / S-row), so sT = W_aug^T @ ctxT_aug exactly.
    KC1 = KC + 1
    xT_d = nc.dram_tensor("xT", [P, KN, BS], F16, kind="ExternalInput").ap()
    ctxT_d = nc.dram_tensor("ctxT", [P, KC1, BS], F16, kind="ExternalInput").ap()
    U_d = nc.dram_tensor("U", [P, KN, RANK], F8, kind="ExternalInput").ap()
    W_d = nc.dram_tensor("W", [P, KC1, RANK], F16, kind="ExternalInput").ap()
    V3_d = nc.dram_tensor("V3", [P, UM, RM, P], F16, kind="ExternalInput").ap()
    if not zero_bias:
        bias_d = nc.dram_tensor("bias", [P, UM], F32, kind="ExternalInput").ap()
    outT_d = nc.dram_tensor("outT", [P, UM, BS], F16, kind="ExternalOutput").ap()

    with tile.TileContext(nc) as tc:
        with (
            tc.tile_pool(name="consts", bufs=1) as cpool,
            tc.tile_pool(name="ctxp", bufs=1) as ctxpool,
            tc.tile_pool(name="wp", bufs=1) as wpool,
            tc.tile_pool(name="xp", bufs=1) as xpool,
            tc.tile_pool(name="up", bufs=1) as upool,
            tc.tile_pool(name="vp", bufs=1) as vpool,
            tc.tile_pool(name="actp", bufs=1) as actpool,
            tc.tile_pool(name="oap", bufs=1) as oa_pool,
            tc.tile_pool(name="odp", bufs=1) as od_pool,
        ):
            # PSUM pools are phase-scoped: mm2+mm1 use 4+4 banks, released
            # before mm3 opens an 8-deep eviction pipeline.
            ps_s_pool = tc.alloc_tile_pool(name="pss", bufs=4, space="PSUM")
            ps_xu_pool = tc.alloc_tile_pool(name="psxu", bufs=4, space="PSUM")

            # Zero tile for the ACT ReLU bias (replaces the suppressed
            # framework const tiles).  DVE memset, pre-touched on ACT.
            zb = cpool.tile([P, 1], F32, name="zb")
            nc.vector.memset(zb[:], 0.0)

            ctx_sb = ctxpool.tile([P, KC1, BS], F16, name="ctx_sb")
            w_sb = wpool.tile([P, KC1, RANK], F16, name="w_sb")

            x_of_kn: dict = {}
            u_of_kn: dict = {}

            def load_x(lo, hi):
                t = xpool.tile([P, hi - lo, BS], F16, name=f"x{lo}")
                nc.sync.dma_start(t[:], xT_d[:, lo:hi, :])
                for kn in range(lo, hi):
                    x_of_kn[kn] = t[:, kn - lo, :]

            def load_u(lo, hi):
                t = upool.tile([P, hi - lo, RANK], F8, name=f"u{lo}")
                nc.scalar.dma_start(t[:], U_d[:, lo:hi, :])
                for kn in range(lo, hi):
                    u_of_kn[kn] = t[:, kn - lo, :]

            # U chunks on the ACT ring (parallel with SP's x stream): the
            # two rings' descgens overlap, and their transfers fair-share
            # the ~0.42 MB/us HBM pipe, so the two streams interleave in
            # need-order.  ctx+W ride the SP stream right after the head
            # chunks.  V is NOT issued here: in-flight DMAs steal
            # bandwidth continuously (measured fair-share, not FIFO), so
            # the V descgens are emitted after mm1 behind a dummy store
            # that waits on sT[0] - they only start once the x/U + ctx/W
            # stream is nearly drained.
            for lo, hi in [(0, 2), (2, 8), (8, 18), (18, 32)]:
                load_u(lo, hi)
            load_x(0, 2)
            load_x(2, 8)
            load_x(8, 18)
            nc.sync.dma_start(ctx_sb[:], ctxT_d[:])
            nc.sync.dma_start(w_sb[:], W_d[:])
            load_x(18, 32)

            if not zero_bias:
                b_sb = cpool.tile([P, UM], F32, name="b_sb")
                nc.scalar.dma_start(b_sb[:], bias_d[:])

            # ---- PE warm-up from the first PE slot during the DMA fill ----
            # warm_src is a raw (non-Tile) scratch region, never written:
            # values are irrelevant and results discarded, so the warmup
            # matmuls carry NO waits and dispatch the moment the PE
            # sequencer enters the body.  Alternate 2 banks so PSUM commit
            # latency doesn't open gaps.
            warm_src = nc.alloc_sbuf_tensor("warm_src", [P, BS + P], F16).ap()
            if sim_memset:
                # CoreSim refuses uninitialized reads; HW doesn't care.
                nc.vector.memset(warm_src[:], 0.0)
            act_scr = cpool.tile([P, 1], F16, name="act_scr")
            ps_warm = [
                ps_s_pool.tile([P, BS], F32, name=f"ps_warm{i}", tag="s")
                for i in range(2)
            ]
            for i in range(N_WARM_MM):
                nc.tensor.matmul(
                    ps_warm[i % 2][:], lhsT=warm_src[:, BS:], rhs=warm_src[:, :BS],
                    start=True, stop=True,
                )
            dve_scr = cpool.tile([P, RM], F32, name="dve_scr")
            # ACT pre-touches: zb (so later ACT ops' bias dep is ACT-local)
            # and bias if present.
            act_zb_scr = cpool.tile([P, 1], F32, name="act_zb_scr")
            nc.scalar.copy(act_zb_scr[:], zb[:])
            if not zero_bias:
                dve_scr2 = cpool.tile([P, UM], F32, name="dve_scr2")
                nc.vector.tensor_copy(dve_scr2[:], b_sb[:])
                act_scr2 = cpool.tile([P, UM], F32, name="act_scr2")
                nc.scalar.copy(act_scr2[:], b_sb[:])

            # ---- mm2 head: kn 0..MM2_HEAD as x/U chunks land ----
            ps_xu = [
                ps_xu_pool.tile([P, BS], F32, name=f"ps_xu{rm}", tag="xu")
                for rm in range(RM)
            ]

            def mm2_chunk(lo, hi):
                for kn in range(lo, hi):
                    ut = u_of_kn[kn]
                    xt = x_of_kn[kn]
                    for rm in range(RM):
                        nc.tensor.matmul(
                            ps_xu[rm][:],
                            lhsT=ut[:, rm * P : (rm + 1) * P],
                            rhs=xt,
                            start=(kn == 0),
                            stop=(kn == KN - 1),
                        )

            mm2_chunk(0, MM2_HEAD)

            # A couple of dep-free garbage matmuls absorb supply jitter
            # between the head and ctx/W's observation, keeping the PE
            # dense for the HAM governor.
            ps_g = ps_s_pool.tile([P, BS], F32, name="ps_g", tag="s")
            for _ in range(2):
                nc.tensor.matmul(
                    ps_g[:], lhsT=warm_src[:, BS:], rhs=warm_src[:, :BS],
                    start=True, stop=True,
                )

            # ---- mm1 (mid-stream; rm-outer so each ps_s[rm] bank stops
            # ~1us apart and the serial DVE sT copies pipeline behind it) ----
            sT = [actpool.tile([P, BS], F32, name=f"sT{rm}") for rm in range(RM)]
            ps_s = [
                ps_s_pool.tile([P, BS], F32, name=f"ps_s{rm}", tag="s")
                for rm in range(RM)
            ]
            for rm in range(RM):
                for kc in range(KC1):
                    nc.tensor.matmul(
                        ps_s[rm][:],
                        lhsT=w_sb[:, kc, rm * P : (rm + 1) * P],
                        rhs=ctx_sb[:, kc, :],
                        start=(kc == 0),
                        stop=(kc == KC1 - 1),
                    )
                nc.vector.tensor_copy(sT[rm][:], ps_s[rm][:])
            # ACT Relu table warm-up here so its table DMA doesn't clog the
            # input stream head.
            nc.scalar.activation(
                act_scr[:], warm_src[:, :1],
                mybir.ActivationFunctionType.Relu, bias=zb[:, :1],
            )

            # ---- V loads on the ACT ring, gated behind an ACT observer
            # copy of sT[0] (~19us): their transfers then don't steal HBM
            # bandwidth from the x/U + ctx/W stream, and each descgen
            # carries only its DMA-lane wait (the data wait lives on the
            # observer copy - walrus encodes at most one wait per DMA). ----
            vth_scr = cpool.tile([P, 1], F32, name="vth_scr")
            nc.scalar.copy(vth_scr[:], sT[0][:, :1])
            vt_of_um: dict = {}
            for v_lo, v_hi in [(0, 8), (8, 20), (20, 32)]:
                vt = vpool.tile([P, v_hi - v_lo, RM, P], F16, name=f"v{v_lo}")
                nc.scalar.dma_start(vt[:], V3_d[:, v_lo:v_hi, :, :])
                for um in range(v_lo, v_hi):
                    vt_of_um[um] = vt[:, um - v_lo, :, :]

            # ---- mm2 mid: kn-outer while the x/U tail streams in ----
            mm2_chunk(MM2_HEAD, KN - 8)
            # ---- mm2 tail rm-OUTER: each ps_xu[rm] bank stops ~0.87us
            # apart, so the serial DVE tT multiplies pipeline behind mm2's
            # last matmuls and tT[3] is ready right as mm3's first group
            # needs it. ----
            for rm in range(RM):
                for kn in range(KN - 8, KN):
                    nc.tensor.matmul(
                        ps_xu[rm][:],
                        lhsT=u_of_kn[kn][:, rm * P : (rm + 1) * P],
                        rhs=x_of_kn[kn],
                        start=False,
                        stop=(kn == KN - 1),
                    )
            tT = [actpool.tile([P, BS], F16, name=f"tT{rm}") for rm in range(RM)]
            # DVE fence: observe sT3's completion tick on DVE so the tT
            # multiplies need only their PE wait (walrus encodes at most one
            # sync wait on DVE tensor ops).
            nc.vector.tensor_copy(dve_scr[:, :1], sT[RM - 1][:, :1])
            for rm in range(RM):
                nc.vector.tensor_mul(tT[rm][:], ps_xu[rm][:], sT[rm][:])

            ps_xu_pool.release()
            ps_s_pool.release()
            ps_o_pool = tc.alloc_tile_pool(name="pso", bufs=8, space="PSUM")

            # Phase-boundary fences: the released PSUM banks carry accessor
            # deps (PE drains, DVE tT reads) into mm3's first ops.  One
            # single-wait fence per engine absorbs them so every mm3
            # instruction keeps <=1 wait.  The anchor must be tT[3] (DVE's
            # LAST tick): mm3 matmuls carry V-chunk + recycled-bank waits
            # already, and an earlier anchor pushes them to 3 waits, which
            # walrus rejects ("Too many sync wait commands").
            nc.tensor.ldweights(tT[RM - 1][:, :P])
            ps_fence = ps_o_pool.tile([P, BS], F32, name="ps_fence", tag="pso")
            nc.tensor.matmul(
                ps_fence[:], lhsT=warm_src[:, BS:], rhs=warm_src[:, :BS],
                start=True, stop=True,
            )
            nc.vector.tensor_copy(dve_scr[:, 1:2], tT[RM - 1][:, :1])
            act_fence_scr = cpool.tile([P, 1], F16, name="act_fence_scr")
            nc.scalar.copy(act_fence_scr[:], tT[RM - 1][:, :1])

            # ---- mm3: outT[um] = relu(Vt[um] @ tT + bias[um]) ----
            # Evictions are DVE-major (ACT also carries the observer copies
            # and all store descgens at ~0.62us each, and must stay under
            # mm3's window).  Each group's first 3/4 ums evict on DVE into
            # og_d, the rest on ACT into og_a; both blocks are contiguous
            # in um so each needs one store.  Stores go on the ACT HWDGE
            # ring, each preceded by a tiny ACT "observer" copy of the
            # source tile: the ACT sequencer then already holds the data
            # tick, so the store itself needs only its DMA-lane wait
            # (walrus encodes at most one sync wait per DMA instruction).
            group_sizes = [16, 8, 4, 2, 1, 1]
            assert sum(group_sizes) == UM
            um0 = 0
            for g, gs in enumerate(group_sizes):
                # Small tail groups evict entirely on DVE (one store, no
                # RELU) so the ACT queue can't straggle past the last
                # matmul.
                d_cnt = gs if gs <= 2 else max(1, (3 * gs) // 4)
                a_cnt = gs - d_cnt
                og_d = od_pool.tile([P, d_cnt, BS], F16, name=f"ogd{g}")
                og_a = (
                    oa_pool.tile([P, a_cnt, BS], F16, name=f"oga{g}")
                    if a_cnt
                    else None
                )
                obs_d = cpool.tile([P, 1], F16, name=f"obsd{g}")
                for j in range(gs):
                    um = um0 + j
                    ps_o = ps_o_pool.tile([P, BS], F32, name="ps_o", tag="pso")
                    vt = vt_of_um[um]  # [P, RM, P]
                    for kr in range(RM):
                        nc.tensor.matmul(
                            ps_o[:],
                            lhsT=vt[:, kr, :],
                            rhs=tT[kr][:],
                            start=(kr == 0),
                            stop=(kr == RM - 1),
                        )
                    if zero_bias:
                        if j < d_cnt:
                            nc.vector.tensor_scalar_max(
                                og_d[:, j, :], ps_o[:], 0.0
                            )
                        else:
                            nc.scalar.activation(
                                og_a[:, j - d_cnt, :], ps_o[:],
                                mybir.ActivationFunctionType.Relu,
                                bias=zb[:, :1],
                            )
                    else:
                        if j < d_cnt:
                            nc.vector.tensor_tensor(
                                og_d[:, j, :], ps_o[:],
                                b_sb[:, um : um + 1].to_broadcast((P, BS)),
                                mybir.AluOpType.add,
                            )
                            nc.vector.tensor_scalar_max(
                                og_d[:, j, :], og_d[:, j, :], 0.0
                            )
                        else:
                            nc.scalar.activation(
                                og_a[:, j - d_cnt, :], ps_o[:],
                                mybir.ActivationFunctionType.Relu,
                                bias=b_sb[:, um : um + 1],
                            )
                nc.scalar.copy(obs_d[:], og_d[:, d_cnt - 1, :1])
                nc.scalar.dma_start(outT_d[:, um0 : um0 + d_cnt, :], og_d[:])
                if a_cnt:
                    obs_a = cpool.tile([P, 1], F16, name=f"obsa{g}")
                    nc.scalar.copy(obs_a[:], og_a[:, a_cnt - 1, :1])
                    nc.scalar.dma_start(
                        outT_d[:, um0 + d_cnt : um0 + gs, :], og_a[:]
                    )
                um0 += gs

            ps_o_pool.release()

    return nc


def _pack_inputs(inputs, context, U, S, V, W, bias):
    """Shard + pack the full fp32 inputs into per-core [128,...] fp16 layouts.

    S is folded into the mm1 operands: ctxT gets a 9th contraction tile that
    is a ones-row (partition 0 only), W gets a matching row carrying S, so
    sT = W_aug^T @ ctxT_aug = S + W^T @ ctxT exactly.
    """
    zero_bias = not bias.any()
    x16 = inputs.astype(np.float16)
    c16 = context.astype(np.float16)
    # U scaled by USCALE into e3m4's normal range; W/S carry 1/USCALE so
    # sT = (S + ctx@W)/USCALE and tT = (USCALE*xu)*(sT) is exact.
    U8 = (U * USCALE).astype(ml_dtypes.float8_e3m4)
    U_pk = np.ascontiguousarray(U8.reshape(KN, P, RANK).transpose(1, 0, 2))
    W_pk = np.zeros((P, KC + 1, RANK), dtype=np.float16)
    W_pk[:, :KC, :] = (
        (W / USCALE).astype(np.float16).reshape(KC, P, RANK).transpose(1, 0, 2)
    )
    W_pk[0, KC, :] = (S / USCALE).astype(np.float16)
    # V3[p, um, kr, c] = V[um*128 + c, kr*128 + p]
    V3_pk = np.ascontiguousarray(
        V.astype(np.float16).reshape(UM, P, RM, P).transpose(3, 0, 2, 1)
    )
    b_pk = np.ascontiguousarray(bias.astype(np.float32).reshape(UM, P).T)

    in_maps = []
    for c in range(M):
        xs = x16[c * BS : (c + 1) * BS]  # [BS, N]
        cs = c16[c * BS : (c + 1) * BS]  # [BS, C]
        xT = np.ascontiguousarray(xs.T.reshape(KN, P, BS).transpose(1, 0, 2))
        ctxT = np.zeros((P, KC + 1, BS), dtype=np.float16)
        ctxT[:, :KC, :] = cs.T.reshape(KC, P, BS).transpose(1, 0, 2)
        ctxT[0, KC, :] = 1.0
        im = {"xT": xT, "ctxT": ctxT, "U": U_pk, "W": W_pk, "V3": V3_pk}
        if not zero_bias:
            im["bias"] = b_pk
        in_maps.append(im)
    return in_maps


_PROGRAM_CACHE = {}


def _get_program(zero_bias: bool) -> bass.Bass:
    if zero_bias not in _PROGRAM_CACHE:
        _PROGRAM_CACHE[zero_bias] = build_program(zero_bias=zero_bias)
    return _PROGRAM_CACHE[zero_bias]


def _unpack_outputs(results) -> np.ndarray:
    shards = []
    for r in results:
        outT = r["outT"]  # [P, UM, BS] fp16
        shards.append(outT.transpose(1, 0, 2).reshape(UNITS, BS).T)
    return np.concatenate(shards, axis=0).astype(np.float32)


def kernel(inputs, context, U, S, V, W, bias, _trace=False):
    bias = np.asarray(bias)
    in_maps = _pack_inputs(
        np.asarray(inputs), np.asarray(context), np.asarray(U),
        np.asarray(S), np.asarray(V), np.asarray(W), bias,
    )
    nc = _get_program(zero_bias=not bias.any())
    res = run_bass_kernel_spmd(nc, in_maps, core_ids=list(range(M)), trace=_trace)
    out = _unpack_outputs(res.results)
    if _trace:
        return out, res
    return out


# revision 17
# speedup vs baseline: 1.1607x; 1.1607x over previous
"""Trainium2 Bass kernel for nn_CADenseAdd (context-adaptive low-rank dense + ReLU).

Reference math (per batch row b):
    s_b   = S + context_b @ W                  # [RANK]
    out_b = relu((x_b @ U) * s_b @ V.T + bias) # [UNITS]

Sharding: data-parallel over batch B=2048 across 8 cores (256 rows/core);
U/S/V/W replicated.  All matmuls are done "transposed" so the contraction
dim always lands on SBUF partitions with zero on-device transposes:

    sT  = W_aug^T @ ctxT_aug      [RANK,  BS]  (S folded in on the host)
    xuT = U^T @ xT                [RANK,  BS]
    tT  = xuT * sT  (cast fp16)   [RANK,  BS]
    outT[um] = Vt[um] @ tT        [UNITS, BS]  (+bias, ReLU on eviction)

The host packs every operand into [128, ...] partition-major contiguous
layout; x/ctx/W/V are fp16, U is fp8-e3m4 with the scale folded into W/S
(end-to-end rel err ~1.24e-2 vs the 2e-2 gate).  PSUM accumulation is
fp32, the elementwise xu*s is fp32.  fp8 double-pumping was measured
numerically infeasible (any single e4m3 operand alone costs ~2.7e-2).

Schedule notes (v2, ~45us target; see measured-timeline derivation below):
- The measured exec window is [first useful body instruction, end of the
  LAST queue's program INCLUDING the runtime's fixed per-queue postamble
  (~7us: each queue zeroes its ~51-semaphore partition of the 256-entry
  file)].  The runtime preamble (~6.4us of barriers) is NOT counted.
- Supply is the first-half wall: ~9.7MB of inputs at the observed
  ~0.42MB/us HBM rate, with ~2.5-3.5us DMA-completion observation
  latency.  Loads stream in strict need-order: x chunks on the SP ring
  and U chunks on the ACT ring (parallel descgen), then ctx+W, x tail,
  V.  mm1 (ctx@W) runs LATE - after a 24-kn mm2 head - so its operands
  arrive with margin (the old early-mm1 schedule stalled the PE 1.35us).
- Garbage warmup matmuls from the first PE slot release the HAM
  clock-gate (1.2 -> 2.4 GHz after ~3.9us of dense PE activity), timed
  to end right as the first x/U chunks' completions are observed.
- Bass's init-time const memsets + all-engine barrier are suppressed
  (the ACT ReLU bias reads an explicitly zeroed tile instead), starting
  the body ~1.2us earlier.
- Teardown is minimal: a chain of single-wait SP nops observes every
  proc's final tick (so the SP queue - whose semaphore partition holds
  ALL of this program's semaphores, re-homed to [207,256) - ends last),
  then nothing: no barrier, no in-program semaphore clears.  The
  runtime's own postamble DRAINs each queue's DMA rings and zeroes every
  semaphore, and the early-finishing queues' postambles overlap the PE
  stream instead of serializing after it.
"""

import re

import ml_dtypes
import numpy as np

import bass_rust
import concourse.bass as bass
import concourse.tile as tile
from concourse import mybir
from concourse.bass_utils import run_bass_kernel_spmd
from concourse.vector_clock import ScopedClock


def _minimal_drain_and_barrier(self, tick_clock, wait_clock):
    """Replacement for TileContext._drain_and_barrier.

    Emit one single-wait SP nop per active proc (this walrus build cannot
    encode >1 sync wait per instruction), so the SP queue observes every
    proc's final tick - including the output stores' DMA-lane semaphores -
    before its program ends.  Everything else (cross-engine barrier,
    in-program semaphore clears, DMA ring resets) is omitted: the runtime's
    per-queue postamble DRAINs the rings and zeroes the entire semaphore
    file anyway, and omitting the barrier lets the other queues' fixed
    ~7us postambles run concurrently with the tail of the compute stream.
    All program semaphores live in the SP queue's clear-partition
    [207, 256), so no other queue's postamble can zero a live semaphore.
    """
    ticks = [int(x) for x in re.findall(r"\d+", repr(tick_clock.global_clock))]
    for proc, tick in enumerate(ticks):
        if tick > 0:
            nop_inst = self.nc.sync.nop(nofuse=True)
            sub = bass_rust.VectorClock()
            sub.require_at_least(proc, tick)
            wait_clock.add_sem_waits(nop_inst.ins, ScopedClock({None: sub}))
    # Python-side bookkeeping of clear_and_free_semaphores, minus the
    # gpsimd dma_reset/sem_clear instructions.
    popped = self.nc._tile_sem_poison_stack.pop()
    assert popped is self._sem_poison
    sems = list(self.sems.allocated().values())
    nums = [s.num if hasattr(s, "num") else int(s) for s in sems]
    self.nc._state.prepend_free_semaphores(nums)
    for poison in self.nc._tile_sem_poison_stack:
        poison.update(nums)


tile.TileContext._drain_and_barrier = _minimal_drain_and_barrier

# Problem shape (hardcoded per contract)
M = 8  # cores
B, N, C = 2048, 4096, 1024
UNITS, RANK = 4096, 512
BS = B // M  # 256 rows per core
P = 128
KN = N // P      # 32 contraction tiles for x @ U
KC = C // P      # 8 contraction tiles for ctx @ W
RM = RANK // P   # 4 tiles of RANK
UM = UNITS // P  # 32 tiles of UNITS

F16 = mybir.dt.float16
F32 = mybir.dt.float32
F8 = mybir.dt.float8e3  # e3m4: 4 mantissa bits

# U ships as fp8-e3m4 scaled by USCALE (entries ~N(0,1/64) land in e3m4's
# normal range); the inverse scale is folded into W and S on the host, so
# tT = (64*xu) * (s/64) is exact.  Halves U's DMA bytes (4MB -> 2MB per
# core) on the supply-bound pre-mm2 stream; end-to-end metric ~1.24e-2
# (deterministic for the harness seed) vs the 2e-2 gate.
USCALE = 64.0

# Warmups bridge [body start ~6.4us, first x/U chunk observed ~10.3-11.4us]
# at the cold 1.2GHz rate (~211ns each) and release the HAM clock-gate.
# Undershooting leaves a PE idle gap that PAUSES the HAM ramp accumulation
# (measured: 20 warmups -> full clock at 12.4us vs 24 -> 11.5us).
N_WARM_MM = 21

# mm1 runs AFTER all of mm2: ctx+W are the last items on the supply
# stream (observed ~23us at the ~0.42MB/us pipe), arriving ~2.5us before
# mm1's ~25.6us start, so supply jitter never stalls the PE mid-stream.


def _make_bass() -> bass.Bass:
    """Bass with the init-time const-tile memsets and all-engine barrier
    suppressed (nothing in this program reads the const tiles - the ACT
    ReLU bias uses an explicit zeroed tile), and the semaphore free-pool
    re-homed into [207, 256): the partition of the semaphore file that the
    runtime postamble of the SP queue (the last queue to finish) clears.
    """
    orig_memset = bass.BassEitherVectorEngine.memset
    orig_aeb = bass.Bass.all_engine_barrier

    def _no_memset(self, ap, constant):
        return None

    def _no_aeb(self, sem_only=False):
        return None

    bass.BassEitherVectorEngine.memset = _no_memset
    bass.Bass.all_engine_barrier = _no_aeb
    try:
        nc = bass.Bass(
            "TRN2",
            debug=False,
            enable_asserts=False,
            enable_partition_id=False,
            dynamic_dma_scratch_size=4096,
        )
    finally:
        bass.BassEitherVectorEngine.memset = orig_memset
        bass.Bass.all_engine_barrier = orig_aeb
    nc._state.reset_free_semaphores(list(range(207, 256)))
    return nc


def build_program(zero_bias: bool = True, sim_memset: bool = False) -> bass.Bass:
    """Build the per-core SPMD program.

    Wait-encoding constraint: this walrus build cannot encode >1 sem-wait
    on DVE/ACT tensor instructions, while matmuls can encode 2.  Every
    DVE/ACT instruction keeps <=1 wait: each engine "pre-touches" its
    DMA-sourced operands once (so later instructions only wait on PE),
    PSUM banks are never shared across phases, and output staging tiles
    are never reused.
    """
    nc = _make_bass()

    # S is folded into mm1 on the host: ctxT/W carry an extra contraction
    # tile (ones-row / S-row), so sT = W_aug^T @ ctxT_aug exactly.
    KC1 = KC + 1
    xT_d = nc.dram_tensor("xT", [P, KN, BS], F16, kind="ExternalInput").ap()
    ctxT_d = nc.dram_tensor("ctxT", [P, KC1, BS], F16, kind="ExternalInput").ap()
    U_d = nc.dram_tensor("U", [P, KN, RANK], F8, kind="ExternalInput").ap()
    W_d = nc.dram_tensor("W", [P, KC1, RANK], F16, kind="ExternalInput").ap()
    V3_d = nc.dram_tensor("V3", [P, UM, RM, P], F16, kind="ExternalInput").ap()
    if not zero_bias:
        bias_d = nc.dram_tensor("bias", [P, UM], F32, kind="ExternalInput").ap()
    outT_d = nc.dram_tensor("outT", [P, UM, BS], F16, kind="ExternalOutput").ap()

    with tile.TileContext(nc) as tc:
        with (
            tc.tile_pool(name="consts", bufs=1) as cpool,
            tc.tile_pool(name="ctxp", bufs=1) as ctxpool,
            tc.tile_pool(name="wp", bufs=1) as wpool,
            tc.tile_pool(name="xp", bufs=1) as xpool,
            tc.tile_pool(name="up", bufs=1) as upool,
            tc.tile_pool(name="vp", bufs=1) as vpool,
            tc.tile_pool(name="actp", bufs=1) as actpool,
            tc.tile_pool(name="oap", bufs=1) as oa_pool,
            tc.tile_pool(name="odp", bufs=1) as od_pool,
        ):
            # PSUM pools are phase-scoped: mm2+mm1 use 4+4 banks, released
            # before mm3 opens an 8-deep eviction pipeline.
            ps_s_pool = tc.alloc_tile_pool(name="pss", bufs=4, space="PSUM")
            ps_xu_pool = tc.alloc_tile_pool(name="psxu", bufs=4, space="PSUM")

            # Zero tile for the ACT ReLU bias (replaces the suppressed
            # framework const tiles).  DVE memset, pre-touched on ACT.
            zb = cpool.tile([P, 1], F32, name="zb")
            nc.vector.memset(zb[:], 0.0)

            ctx_sb = ctxpool.tile([P, KC1, BS], F16, name="ctx_sb")
            w_sb = wpool.tile([P, KC1, RANK], F16, name="w_sb")

            x_of_kn: dict = {}
            u_of_kn: dict = {}

            def load_x(lo, hi):
                t = xpool.tile([P, hi - lo, BS], F16, name=f"x{lo}")
                nc.sync.dma_start(t[:], xT_d[:, lo:hi, :])
                for kn in range(lo, hi):
                    x_of_kn[kn] = t[:, kn - lo, :]

            def load_u(lo, hi):
                t = upool.tile([P, hi - lo, RANK], F8, name=f"u{lo}")
                nc.scalar.dma_start(t[:], U_d[:, lo:hi, :])
                for kn in range(lo, hi):
                    u_of_kn[kn] = t[:, kn - lo, :]

            # U chunks on the ACT ring (parallel with SP's x stream): the
            # two rings' descgens overlap, and their transfers fair-share
            # the ~0.42 MB/us HBM pipe, so the two streams interleave in
            # need-order.  ctx+W ride the SP stream right after the head
            # chunks.  V is NOT issued here: in-flight DMAs steal
            # bandwidth continuously (measured fair-share, not FIFO), so
            # the V descgens are emitted after mm1 behind a dummy store
            # that waits on sT[0] - they only start once the x/U + ctx/W
            # stream is nearly drained.
            for lo, hi in [(0, 2), (2, 8), (8, 18), (18, 32)]:
                load_u(lo, hi)
                load_x(lo, hi)
            # ctx+W land LAST (mm1 runs after all of mm2): the 9th/10th
            # dma wait for the first freed lanes (~10us) and their bytes
            # trail the x/U stream, observed ~23us vs mm1's ~25.6us start.
            nc.sync.dma_start(ctx_sb[:], ctxT_d[:])
            nc.sync.dma_start(w_sb[:], W_d[:])

            if not zero_bias:
                b_sb = cpool.tile([P, UM], F32, name="b_sb")
                nc.scalar.dma_start(b_sb[:], bias_d[:])

            # ---- PE warm-up from the first PE slot during the DMA fill ----
            # warm_src is a raw (non-Tile) scratch region, never written:
            # values are irrelevant and results discarded, so the warmup
            # matmuls carry NO waits and dispatch the moment the PE
            # sequencer enters the body.  Alternate 2 banks so PSUM commit
            # latency doesn't open gaps.
            warm_src = nc.alloc_sbuf_tensor("warm_src", [P, BS + P], F16).ap()
            if sim_memset:
                # CoreSim refuses uninitialized reads; HW doesn't care.
                nc.vector.memset(warm_src[:], 0.0)
            act_scr = cpool.tile([P, 1], F16, name="act_scr")
            ps_warm = [
                ps_s_pool.tile([P, BS], F32, name=f"ps_warm{i}", tag="s")
                for i in range(2)
            ]
            for i in range(N_WARM_MM):
                nc.tensor.matmul(
                    ps_warm[i % 2][:], lhsT=warm_src[:, BS:], rhs=warm_src[:, :BS],
                    start=True, stop=True,
                )
            dve_scr = cpool.tile([P, RM], F32, name="dve_scr")
            # ACT pre-touches: zb (so later ACT ops' bias dep is ACT-local)
            # and bias if present.
            act_zb_scr = cpool.tile([P, 1], F32, name="act_zb_scr")
            nc.scalar.copy(act_zb_scr[:], zb[:])
            if not zero_bias:
                dve_scr2 = cpool.tile([P, UM], F32, name="dve_scr2")
                nc.vector.tensor_copy(dve_scr2[:], b_sb[:])
                act_scr2 = cpool.tile([P, UM], F32, name="act_scr2")
                nc.scalar.copy(act_scr2[:], b_sb[:])

            # ---- mm2 head: kn 0..MM2_HEAD as x/U chunks land ----
            ps_xu = [
                ps_xu_pool.tile([P, BS], F32, name=f"ps_xu{rm}", tag="xu")
                for rm in range(RM)
            ]

            def mm2_chunk(lo, hi):
                for kn in range(lo, hi):
                    ut = u_of_kn[kn]
                    xt = x_of_kn[kn]
                    for rm in range(RM):
                        nc.tensor.matmul(
                            ps_xu[rm][:],
                            lhsT=ut[:, rm * P : (rm + 1) * P],
                            rhs=xt,
                            start=(kn == 0),
                            stop=(kn == KN - 1),
                        )

            mm2_chunk(0, KN - 8)

            # Garbage matmuls at the kn24 point: absorb supply jitter, keep
            # the PE dense for the HAM governor, and provide the PE tick
            # that anchors the V-load throttle below.
            ps_g = ps_s_pool.tile([P, BS], F32, name="ps_g", tag="s")
            for _ in range(2):
                nc.tensor.matmul(
                    ps_g[:], lhsT=warm_src[:, BS:], rhs=warm_src[:, :BS],
                    start=True, stop=True,
                )

            # ---- mm2 tail rm-OUTER: each ps_xu[rm] bank stops ~0.87us
            # apart, so the serial DVE tT multiplies pipeline behind the
            # mm1 chain and tT[3] is ready right as mm3 needs it. ----
            for rm in range(RM):
                for kn in range(KN - 8, KN):
                    nc.tensor.matmul(
                        ps_xu[rm][:],
                        lhsT=u_of_kn[kn][:, rm * P : (rm + 1) * P],
                        rhs=x_of_kn[kn],
                        start=False,
                        stop=(kn == KN - 1),
                    )

            # ---- V loads on the ACT ring, throttled so their transfers
            # don't steal HBM bandwidth from the x/U + ctx/W stream.  The
            # Tile scheduler reorders dep-free DMAs, so the delay must be a
            # REAL dependency chain: one DVE marker instruction - waiting
            # ps_xu[0]'s group close (~25us; its DVE tick later merges into
            # deps the mm3 fence already absorbs) - writes into every V
            # chunk's start region, one ACT observer of the padded um-slot
            # 32 (which no dma overwrites, so no WAR) absorbs the DVE tick,
            # and each chunk's dma_start overwrites its marker: a WAW whose
            # wait elides, leaving each descgen only its lane wait.
            vt_all = vpool.tile([P, UM + 8, RM, P], F16, name="vt_all")
            vt_of_um = {um: vt_all[:, um, :, :] for um in range(UM)}
            V_CHUNKS = [(0, 8), (8, 16), (16, 24), (24, 32)]
            nc.vector.tensor_copy(
                vt_all[:, 0:33:8, 0, :1], ps_xu[0][:, :5, None, None]
            )
            vth_scr = cpool.tile([P, 1], F16, name="vth_scr")
            nc.scalar.copy(vth_scr[:], vt_all[:, 32, 0, :1])
            for v_lo, v_hi in V_CHUNKS:
                nc.scalar.dma_start(
                    vt_all[:, v_lo:v_hi], V3_d[:, v_lo:v_hi, :, :]
                )

            # ---- mm1 (last; rm-outer so each ps_s[rm] bank stops ~1us
            # apart and the serial DVE sT copies pipeline behind it) ----
            sT = [actpool.tile([P, BS], F32, name=f"sT{rm}") for rm in range(RM)]
            ps_s = [
                ps_s_pool.tile([P, BS], F32, name=f"ps_s{rm}", tag="s")
                for rm in range(RM)
            ]
            for rm in range(RM):
                for kc in range(KC1):
                    nc.tensor.matmul(
                        ps_s[rm][:],
                        lhsT=w_sb[:, kc, rm * P : (rm + 1) * P],
                        rhs=ctx_sb[:, kc, :],
                        start=(kc == 0),
                        stop=(kc == KC1 - 1),
                    )
                nc.vector.tensor_copy(sT[rm][:], ps_s[rm][:])
            # ACT Relu table warm-up here so its table DMA doesn't clog the
            # input stream head.
            nc.scalar.activation(
                act_scr[:], warm_src[:, :1],
                mybir.ActivationFunctionType.Relu, bias=zb[:, :1],
            )

            tT = [actpool.tile([P, BS], F16, name=f"tT{rm}") for rm in range(RM)]
            # DVE fence: observe sT3's completion tick on DVE so the tT
            # multiplies need only their PE wait (walrus encodes at most one
            # sync wait on DVE tensor ops).
            nc.vector.tensor_copy(dve_scr[:, :1], sT[RM - 1][:, :1])
            for rm in range(RM):
                nc.vector.tensor_mul(tT[rm][:], ps_xu[rm][:], sT[rm][:])

            ps_xu_pool.release()
            ps_s_pool.release()
            ps_o_pool = tc.alloc_tile_pool(name="pso", bufs=8, space="PSUM")

            # Phase-boundary fences: the released PSUM banks carry accessor
            # deps (PE drains, DVE tT reads) into mm3's first ops.  One
            # single-wait fence per engine absorbs them so every mm3
            # instruction keeps <=1 wait.  The anchor must be tT[3] (DVE's
            # LAST tick): mm3 matmuls carry V-chunk + recycled-bank waits
            # already, and an earlier anchor pushes them to 3 waits, which
            # walrus rejects ("Too many sync wait commands").
            nc.tensor.ldweights(tT[RM - 1][:, :P])
            ps_fence = ps_o_pool.tile([P, BS], F32, name="ps_fence", tag="pso")
            nc.tensor.matmul(
                ps_fence[:], lhsT=warm_src[:, BS:], rhs=warm_src[:, :BS],
                start=True, stop=True,
            )
            nc.vector.tensor_copy(dve_scr[:, 1:2], tT[RM - 1][:, :1])
            act_fence_scr = cpool.tile([P, 1], F16, name="act_fence_scr")
            nc.scalar.copy(act_fence_scr[:], tT[RM - 1][:, :1])

            # ---- mm3: outT[um] = relu(Vt[um] @ tT + bias[um]) ----
            # Evictions are DVE-major (ACT also carries the observer copies
            # and all store descgens at ~0.62us each, and must stay under
            # mm3's window).  Each group's first 3/4 ums evict on DVE into
            # og_d, the rest on ACT into og_a; both blocks are contiguous
            # in um so each needs one store.  Stores go on the ACT HWDGE
            # ring, each preceded by a tiny ACT "observer" copy of the
            # source tile: the ACT sequencer then already holds the data
            # tick, so the store itself needs only its DMA-lane wait
            # (walrus encodes at most one sync wait per DMA instruction).
            group_sizes = [16, 8, 4, 2, 1, 1]
            assert sum(group_sizes) == UM
            um0 = 0
            for g, gs in enumerate(group_sizes):
                # Small tail groups evict entirely on DVE (one store, no
                # RELU) so the ACT queue can't straggle past the last
                # matmul.
                d_cnt = gs if gs <= 2 else max(1, (3 * gs) // 4)
                a_cnt = gs - d_cnt
                og_d = od_pool.tile([P, d_cnt, BS], F16, name=f"ogd{g}")
                og_a = (
                    oa_pool.tile([P, a_cnt, BS], F16, name=f"oga{g}")
                    if a_cnt
                    else None
                )
                obs_d = cpool.tile([P, 1], F16, name=f"obsd{g}")
                for j in range(gs):
                    um = um0 + j
                    ps_o = ps_o_pool.tile([P, BS], F32, name="ps_o", tag="pso")
                    vt = vt_of_um[um]  # [P, RM, P]
                    for kr in range(RM):
                        nc.tensor.matmul(
                            ps_o[:],
                            lhsT=vt[:, kr, :],
                            rhs=tT[kr][:],
                            start=(kr == 0),
                            stop=(kr == RM - 1),
                        )
                    if zero_bias:
                        if j < d_cnt:
                            nc.vector.tensor_scalar_max(
                                og_d[:, j, :], ps_o[:], 0.0
                            )
                        else:
                            nc.scalar.activation(
                                og_a[:, j - d_cnt, :], ps_o[:],
                                mybir.ActivationFunctionType.Relu,
                                bias=zb[:, :1],
                            )
                    else:
                        if j < d_cnt:
                            nc.vector.tensor_tensor(
                                og_d[:, j, :], ps_o[:],
                                b_sb[:, um : um + 1].to_broadcast((P, BS)),
                                mybir.AluOpType.add,
                            )
                            nc.vector.tensor_scalar_max(
                                og_d[:, j, :], og_d[:, j, :], 0.0
                            )
                        else:
                            nc.scalar.activation(
                                og_a[:, j - d_cnt, :], ps_o[:],
                                mybir.ActivationFunctionType.Relu,
                                bias=b_sb[:, um : um + 1],
                            )
                nc.scalar.copy(obs_d[:], og_d[:, d_cnt - 1, :1])
                nc.scalar.dma_start(outT_d[:, um0 : um0 + d_cnt, :], og_d[:])
                if a_cnt:
                    obs_a = cpool.tile([P, 1], F16, name=f"obsa{g}")
                    nc.scalar.copy(obs_a[:], og_a[:, a_cnt - 1, :1])
                    nc.scalar.dma_start(
                        outT_d[:, um0 + d_cnt : um0 + gs, :], og_a[:]
                    )
                um0 += gs

            ps_o_pool.release()

    return nc


def _pack_inputs(inputs, context, U, S, V, W, bias):
    """Shard + pack the full fp32 inputs into per-core [128,...] fp16 layouts.

    S is folded into the mm1 operands: ctxT gets a 9th contraction tile that
    is a ones-row (partition 0 only), W gets a matching row carrying S, so
    sT = W_aug^T @ ctxT_aug = S + W^T @ ctxT exactly.
    """
    zero_bias = not bias.any()
    x16 = inputs.astype(np.float16)
    c16 = context.astype(np.float16)
    # U scaled by USCALE into e3m4's normal range; W/S carry 1/USCALE so
    # sT = (S + ctx@W)/USCALE and tT = (USCALE*xu)*(sT) is exact.
    U8 = (U * USCALE).astype(ml_dtypes.float8_e3m4)
    U_pk = np.ascontiguousarray(U8.reshape(KN, P, RANK).transpose(1, 0, 2))
    W_pk = np.zeros((P, KC + 1, RANK), dtype=np.float16)
    W_pk[:, :KC, :] = (
        (W / USCALE).astype(np.float16).reshape(KC, P, RANK).transpose(1, 0, 2)
    )
    W_pk[0, KC, :] = (S / USCALE).astype(np.float16)
    # V3[p, um, kr, c] = V[um*128 + c, kr*128 + p]
    V3_pk = np.ascontiguousarray(
        V.astype(np.float16).reshape(UM, P, RM, P).transpose(3, 0, 2, 1)
    )
    b_pk = np.ascontiguousarray(bias.astype(np.float32).reshape(UM, P).T)

    in_maps = []
    for c in range(M):
        xs = x16[c * BS : (c + 1) * BS]  # [BS, N]
        cs = c16[c * BS : (c + 1) * BS]  # [BS, C]
        xT = np.ascontiguousarray(xs.T.reshape(KN, P, BS).transpose(1, 0, 2))
        ctxT = np.zeros((P, KC + 1, BS), dtype=np.float16)
        ctxT[:, :KC, :] = cs.T.reshape(KC, P, BS).transpose(1, 0, 2)
        ctxT[0, KC, :] = 1.0
        im = {"xT": xT, "ctxT": ctxT, "U": U_pk, "W": W_pk, "V3": V3_pk}
        if not zero_bias:
            im["bias"] = b_pk
        in_maps.append(im)
    return in_maps


_PROGRAM_CACHE = {}


def _get_program(zero_bias: bool) -> bass.Bass:
    if zero_bias not in _PROGRAM_CACHE:
        _PROGRAM_CACHE[zero_bias] = build_program(zero_bias=zero_bias)
    return _PROGRAM_CACHE[zero_bias]


def _unpack_outputs(results) -> np.ndarray:
    shards = []
    for r in results:
        outT = r["outT"]  # [P, UM, BS] fp16
        shards.append(outT.transpose(1, 0, 2).reshape(UNITS, BS).T)
    return np.concatenate(shards, axis=0).astype(np.float32)


def kernel(inputs, context, U, S, V, W, bias, _trace=False):
    bias = np.asarray(bias)
    in_maps = _pack_inputs(
        np.asarray(inputs), np.asarray(context), np.asarray(U),
        np.asarray(S), np.asarray(V), np.asarray(W), bias,
    )
    nc = _get_program(zero_bias=not bias.any())
    res = run_bass_kernel_spmd(nc, in_maps, core_ids=list(range(M)), trace=_trace)
    out = _unpack_outputs(res.results)
    if _trace:
        return out, res
    return out
